# revision 1
# baseline (speedup 1.0000x reference)
"""LurieNet-k Trainium2 kernel.

Computes, from the raw parametrization tensors, the matrices
  C = UC @ SC @ VC^T,  B = UB @ SB @ VB^T,
  A = 0.5*UA @ SA @ UA^T + 0.5*YA  (SA = -(alpha_upp*I + GA))
entirely on device (matrix exponentials of skew matrices via
scaling-and-squaring Taylor), then runs the 511-step recurrence
  y  = C x + by
  x' = x + (0.01*A x + 0.01*B tanh(y) + 0.01*bx)
on a (128, 64) state shard per NeuronCore (batch data-parallel over the
8 cores), writing the full (b, t, n) trajectory.

Precision: the system amplifies per-step state perturbations, so the
identity part of the state update is carried in fp32 OUTSIDE the
matmuls (split-carry): the matmuls only apply gains << 1 (C ~ 0.3,
0.01A ~ 0.003, 0.01B ~ 0.003), so their operands can be bf16 without
error blowup (measured ~2e-3 final rel err vs fp32's 6e-6; naive bf16
with A' = I + 0.01A as a matmul diverges to 0.36). bf16 weights get
FWL weight loads, which matters because the kernel is otherwise
LDWEIGHTS-throughput bound.
"""

import sys

for _p in ("/opt/trn_rl_repo",):
    if _p not in sys.path:
        sys.path.insert(0, _p)

import numpy as np

import concourse.bass as bass
import concourse.mybir as mybir
import concourse.tile as tile
from concourse import bacc
from concourse import bass_isa
from concourse.bass import ds
from concourse.bass_utils import run_bass_kernel_spmd
from concourse.masks import make_identity, make_upper_triangular

F32 = mybir.dt.float32
F32R = mybir.dt.float32r
BF16 = mybir.dt.bfloat16
ALU = mybir.AluOpType
ACTF = mybir.ActivationFunctionType
AXIS = mybir.AxisListType

N = 128          # state dim
TMAX = 512       # time steps (including t=0)
BS = 512         # global batch
NCORES = 8
BSH = BS // NCORES   # 64 batch columns per core
STEP = 0.01
KTOP = 4

EXPM_SCAL = 3    # expm scaling: X = S / 2**EXPM_SCAL, then 3 squarings
EXPM_TERMS = 4   # Taylor terms in the Horner evaluation

PARAM_NAMES = [
    "ZA_Y", "ZA_U", "ZA_G", "ZB_U", "ZB_V", "ZB_S", "ZC_U", "ZC_V", "ZC_S",
]


def build_program(tmax=TMAX, tc_chunk=32, mdt=BF16):
    """Build the single-NeuronCore Bass program (run SPMD on all 8 cores).

    mdt: dtype of the recurrence matmul operands (weights, rounded state
    copy, tanh output). The fp32 state carry is exact regardless.
    """
    assert tmax % tc_chunk == 0 and tc_chunk % 2 == 0
    half = tc_chunk // 2
    nchunks = tmax // tc_chunk

    nc = bacc.Bacc(
        "TRN2",
        target_bir_lowering=False,
        debug=False,
        enable_asserts=False,
        num_devices=NCORES,
    )

    x0 = nc.dram_tensor("x0", [N, BSH], F32, kind="ExternalInput")
    zs = {
        name: nc.dram_tensor(name, [N, N], F32, kind="ExternalInput")
        for name in PARAM_NAMES
    }
    bx_d = nc.dram_tensor("bx", [N, 1], F32, kind="ExternalInput")
    by_d = nc.dram_tensor("by", [N, 1], F32, kind="ExternalInput")
    out = nc.dram_tensor("out", [BSH, tmax, N], F32, kind="ExternalOutput")

    with tile.TileContext(nc) as tc:
        with tc.tile_pool(name="const", bufs=1) as constp:
            ident = constp.tile([N, N], F32, tag="ident")
            make_identity(nc, ident[:])
            masku = constp.tile([N, N], F32, tag="masku")
            make_upper_triangular(nc, masku[:], val=1.0, diag=False)
            ident_r32 = constp.tile([N, N], F32R, tag="ident_r32")
            nc.vector.tensor_copy(ident_r32[:], ident[:])

            by_c = constp.tile([N, 1], F32, tag="by")
            nc.sync.dma_start(out=by_c[:], in_=by_d[:])
            bx_c = constp.tile([N, 1], F32, tag="bxraw")
            nc.sync.dma_start(out=bx_c[:], in_=bx_d[:])
            bxp_c = constp.tile([N, 1], F32, tag="bxp")
            nc.vector.tensor_scalar_mul(bxp_c[:], bx_c[:], STEP)
            x0_c = constp.tile([N, BSH], F32, tag="x0c")
            nc.sync.dma_start(out=x0_c[:], in_=x0[:])

            # ------- setup phase: expm's + weight assembly -------
            # Recurrence uses the P-form to split the serial chain in two:
            #   y_t  = P x_{t-1} + Q th_{t-1} + r,  th_t = tanh(y_t)
            #   x_t  = x_{t-1} + (0.01A) x_{t-1} + (0.01B) th_{t-1} + 0.01 bx
            # with P = C + 0.01 C A, Q = 0.01 C B, r = 0.01 C bx + by.
            PTm = constp.tile([N, N], mdt, tag="PTm")      # P^T
            QTm = constp.tile([N, N], mdt, tag="QTm")      # Q^T
            A01Tm = constp.tile([N, N], mdt, tag="A01Tm")  # (0.01 A)^T
            BpTm = constp.tile([N, N], mdt, tag="BpTm")    # (0.01 B)^T
            CTf32 = constp.tile([N, N], F32, tag="CTf32")  # C^T fp32
            r_c = constp.tile([N, 1], F32, tag="rc")       # 0.01 C bx + by

            with (
                tc.tile_pool(name="zbuf", bufs=1) as zp,
                tc.tile_pool(name="work", bufs=2) as wp,
                tc.tile_pool(name="eres", bufs=1) as ep,
                tc.tile_pool(name="small", bufs=1) as sp,
                tc.tile_pool(name="pss", bufs=4, space="PSUM") as psp,
            ):
                zt = {}
                for name in PARAM_NAMES:
                    zt[name] = zp.tile([N, N], F32, tag=name, name=f"z_{name}")
                    nc.sync.dma_start(out=zt[name][:], in_=zs[name][:])

                def expm_batch(specs):
                    """Interleaved expm(skew(Z))^T for all matrices at once.

                    Maintains the (T, T^T) pair through Horner + squaring so
                    no PE transposes are needed: with negX = X^T = -X,
                      X @ T     = matmul(lhsT=negX, rhs=T)
                      T^T @ X^T = matmul(lhsT=T,    rhs=negX)
                    The five chains are advanced stage-by-stage so PE/DVE/ACT
                    work from different chains overlaps (a single sequential
                    chain is latency-bound on the MM->STT->MM dependency).
                    """
                    scal = 1.0 / (2.0 ** EXPM_SCAL)
                    negx = {}
                    t_cur = {}
                    tt_cur = {}
                    for z_tile, tag in specs:
                        us = wp.tile([N, N], F32R, tag="us_r", name=f"us_{tag}")
                        nc.vector.scalar_tensor_tensor(
                            us[:], z_tile[:], scal, masku[:],
                            op0=ALU.mult, op1=ALU.mult,
                        )
                        pst = psp.tile([N, N], F32R, tag="ps", bufs=8,
                                       name=f"pst_{tag}")
                        nc.tensor.transpose(pst[:], us[:], ident_r32[:])
                        nx = wp.tile([N, N], F32R, tag=f"negx_{tag}", bufs=1,
                                     name=f"negx_{tag}")
                        nc.vector.scalar_tensor_tensor(
                            nx[:], pst[:], 1.0, us[:],
                            op0=ALU.mult, op1=ALU.subtract,
                        )
                        negx[tag] = nx
                        t_cur[tag] = ident_r32
                        tt_cur[tag] = ident_r32
                    # Horner advances only T (the T^T pair is rebuilt by a
                    # PE transpose afterwards -- halves the vector-op load,
                    # which bounds this phase).
                    for j in range(EXPM_TERMS, 0, -1):
                        for _, tag in specs:
                            psa = psp.tile([N, N], F32, tag="ps", bufs=8)
                            nc.tensor.matmul(
                                psa[:], negx[tag][:], t_cur[tag][:],
                                start=True, stop=True,
                            )
                            t_new = wp.tile([N, N], F32R, tag=f"T_{tag}",
                                            bufs=2, name=f"T_{tag}")
                            nc.vector.scalar_tensor_tensor(
                                t_new[:], psa[:], 1.0 / j, ident_r32[:],
                                op0=ALU.mult, op1=ALU.add,
                            )
                            t_cur[tag] = t_new
                    for _, tag in specs:
                        pst = psp.tile([N, N], F32R, tag="ps", bufs=8,
                                       name=f"ptt_{tag}")
                        nc.tensor.transpose(pst[:], t_cur[tag][:], ident_r32[:])
                        tt_new = wp.tile([N, N], F32R, tag=f"TT_{tag}",
                                         bufs=2, name=f"TT_{tag}")
                        nc.scalar.copy(tt_new[:], pst[:])
                        tt_cur[tag] = tt_new
                    for _ in range(EXPM_SCAL):
                        for _, tag in specs:
                            psa = psp.tile([N, N], F32, tag="ps", bufs=8)
                            psb = psp.tile([N, N], F32, tag="ps", bufs=8)
                            nc.tensor.matmul(
                                psa[:], tt_cur[tag][:], t_cur[tag][:],
                                start=True, stop=True,
                            )
                            nc.tensor.matmul(
                                psb[:], t_cur[tag][:], tt_cur[tag][:],
                                start=True, stop=True,
                            )
                            t_new = wp.tile([N, N], F32R, tag=f"T_{tag}",
                                            bufs=2, name=f"T_{tag}")
                            tt_new = wp.tile([N, N], F32R, tag=f"TT_{tag}",
                                             bufs=2, name=f"TT_{tag}")
                            nc.vector.tensor_copy(t_new[:], psa[:])
                            nc.scalar.copy(tt_new[:], psb[:])
                            t_cur[tag], tt_cur[tag] = t_new, tt_new
                    return tt_cur

                eres = expm_batch([
                    (zt["ZC_U"], "UCT"), (zt["ZC_V"], "VCT"),
                    (zt["ZB_U"], "UBT"), (zt["ZB_V"], "VBT"),
                    (zt["ZA_U"], "UAT"),
                ])
                uct, vct = eres["UCT"], eres["VCT"]
                ubt, vbt = eres["UBT"], eres["VBT"]
                uat = eres["UAT"]

                def absdiag_col(z_tile, tag):
                    tmp = wp.tile([N, N], F32, tag="us")
                    nc.vector.tensor_mul(tmp[:], z_tile[:], ident[:])
                    col = sp.tile([N, 1], F32, tag=tag, name=f"col_{tag}")
                    nc.vector.tensor_reduce(
                        col[:], tmp[:], AXIS.X, ALU.add, apply_absolute_value=True
                    )
                    return col

                dc_col = absdiag_col(zt["ZC_S"], "dc")   # |diag(ZC_S)|
                db_col = absdiag_col(zt["ZB_S"], "db")   # |diag(ZB_S)|
                ga_col = absdiag_col(zt["ZA_G"], "ga")   # |diag(ZA_G)|

                # top-4: alpha = sqrt(sum_i (b_i c_i)^2), b/c sorted desc.
                bwork = sp.tile([N, 1], F32, tag="bwork")
                cwork = sp.tile([N, 1], F32, tag="cwork")
                nc.vector.tensor_copy(bwork[:], db_col[:])
                nc.vector.tensor_copy(cwork[:], dc_col[:])
                acc = sp.tile([N, 1], F32, tag="acc")
                nc.vector.memset(acc[:], 0.0)
                bmax = sp.tile([N, 1], F32, tag="bmax")
                cmax = sp.tile([N, 1], F32, tag="cmax")
                prod = sp.tile([N, 1], F32, tag="prod")
                gmask = sp.tile([N, 1], F32, tag="gmask")
                tdrop = sp.tile([N, 1], F32, tag="tdrop")
                for i in range(KTOP):
                    nc.gpsimd.partition_all_reduce(
                        bmax[:], bwork[:], N, bass_isa.ReduceOp.max
                    )
                    nc.gpsimd.partition_all_reduce(
                        cmax[:], cwork[:], N, bass_isa.ReduceOp.max
                    )
                    nc.vector.tensor_mul(prod[:], bmax[:], cmax[:])
                    nc.vector.tensor_mul(prod[:], prod[:], prod[:])
                    nc.vector.tensor_add(acc[:], acc[:], prod[:])
                    if i < KTOP - 1:
                        # zero out the extracted max (values all > 0)
                        nc.vector.tensor_single_scalar(
                            gmask[:], bwork[:], bmax[:], ALU.is_ge
                        )
                        nc.vector.tensor_mul(tdrop[:], bwork[:], gmask[:])
                        nc.vector.tensor_sub(bwork[:], bwork[:], tdrop[:])
                        nc.vector.tensor_single_scalar(
                            gmask[:], cwork[:], cmax[:], ALU.is_ge
                        )
                        nc.vector.tensor_mul(tdrop[:], cwork[:], gmask[:])
                        nc.vector.tensor_sub(cwork[:], cwork[:], tdrop[:])
                alpha = sp.tile([N, 1], F32, tag="alpha")
                nc.scalar.activation(alpha[:], acc[:], ACTF.Sqrt)

                # sa05 = -0.5*(alpha + gA)  (per-partition row scale of UA^T)
                sa05 = sp.tile([N, 1], F32, tag="sa05")
                nc.vector.tensor_scalar(
                    sa05[:], ga_col[:], alpha[:], -0.5, op0=ALU.add, op1=ALU.mult
                )
                sb01 = sp.tile([N, 1], F32, tag="sb01")
                nc.vector.tensor_scalar_mul(sb01[:], db_col[:], STEP)

                # C^T = VC @ (SC @ UC^T)
                p1 = wp.tile([N, N], F32R, tag="us_r", name="p1")
                nc.vector.tensor_scalar_mul(p1[:], uct[:], dc_col[:])
                psa = psp.tile([N, N], F32, tag="ps", bufs=8)
                nc.tensor.matmul(psa[:], vct[:], p1[:], start=True, stop=True)
                nc.vector.tensor_copy(CTf32[:], psa[:])

                # (0.01 B)^T = VB @ (0.01 SB @ UB^T)
                p2 = wp.tile([N, N], F32R, tag="us_r", name="p2")
                nc.vector.tensor_scalar_mul(p2[:], ubt[:], sb01[:])
                psb = psp.tile([N, N], F32, tag="ps", bufs=8)
                nc.tensor.matmul(psb[:], vbt[:], p2[:], start=True, stop=True)
                nc.vector.tensor_copy(BpTm[:], psb[:])
                # untransposed 0.01 B = UB @ (0.01 SB @ VB^T)
                p2b = wp.tile([N, N], F32R, tag="us_r", name="p2b")
                nc.vector.tensor_scalar_mul(p2b[:], vbt[:], sb01[:])
                psb2 = psp.tile([N, N], F32, tag="ps", bufs=8)
                nc.tensor.matmul(psb2[:], ubt[:], p2b[:], start=True, stop=True)
                bp_un = ep.tile([N, N], F32, tag="Bpun")
                nc.vector.tensor_copy(bp_un[:], psb2[:])

                # M = UA @ (sa05 * UA^T) = 0.5*UA SA UA^T (symmetric)
                p3 = wp.tile([N, N], F32R, tag="us_r", name="p3")
                nc.vector.tensor_scalar_mul(p3[:], uat[:], sa05[:])
                psm = psp.tile([N, N], F32, tag="ps", bufs=8)
                nc.tensor.matmul(psm[:], uat[:], p3[:], start=True, stop=True)
                # YA = Uy - Uy^T; q2 = -0.005*YA
                uy = wp.tile([N, N], F32, tag="us")
                nc.vector.tensor_mul(uy[:], zt["ZA_Y"][:], masku[:])
                pst2 = psp.tile([N, N], F32, tag="ps", bufs=8)
                nc.tensor.transpose(pst2[:], uy[:], ident[:])
                nc.vector.tensor_scalar_mul(uy[:], uy[:], 0.5 * STEP)
                q2 = wp.tile([N, N], F32, tag="T")
                nc.vector.scalar_tensor_tensor(
                    q2[:], pst2[:], 0.5 * STEP, uy[:], op0=ALU.mult, op1=ALU.subtract
                )
                # (0.01 A)^T = 0.01*M + q2 ; untransposed 0.01 A = 0.01*M - q2
                nc.vector.scalar_tensor_tensor(
                    A01Tm[:], psm[:], STEP, q2[:], op0=ALU.mult, op1=ALU.add
                )
                a01_un = ep.tile([N, N], F32, tag="A01un")
                nc.vector.scalar_tensor_tensor(
                    a01_un[:], psm[:], STEP, q2[:], op0=ALU.mult, op1=ALU.subtract
                )

                # P^T = C^T + (0.01 A)^T C^T ;  Q^T = (0.01 B)^T C^T
                psw = psp.tile([N, N], F32, tag="ps", bufs=8)
                nc.tensor.matmul(psw[:], a01_un[:], CTf32[:], start=True, stop=True)
                nc.vector.scalar_tensor_tensor(
                    PTm[:], psw[:], 1.0, CTf32[:], op0=ALU.mult, op1=ALU.add
                )
                psq = psp.tile([N, N], F32, tag="ps", bufs=8)
                nc.tensor.matmul(psq[:], bp_un[:], CTf32[:], start=True, stop=True)
                nc.vector.tensor_copy(QTm[:], psq[:])

                # r = 0.01 C bx + by
                psr = psp.tile([N, 1], F32, tag="ps", bufs=8, name="psr")
                nc.tensor.matmul(psr[:], CTf32[:], bxp_c[:], start=True, stop=True)
                nc.vector.scalar_tensor_tensor(
                    r_c[:], psr[:], 1.0, by_c[:], op0=ALU.mult, op1=ALU.add
                )

            # ------- recurrence (split-carry) -------
            with (
                tc.tile_pool(name="xbuf", bufs=2) as xbufp,
                tc.tile_pool(name="stage", bufs=2) as stagep,
                tc.tile_pool(name="xr", bufs=3) as xrp,
                tc.tile_pool(name="th", bufs=3) as thp,
                tc.tile_pool(name="psy", bufs=3, space="PSUM") as psyp,
                tc.tile_pool(name="psx", bufs=3, space="PSUM") as psxp,
                tc.tile_pool(name="pstr", bufs=2, space="PSUM") as pstrp,
            ):
                # xbuf column slot for local step s: pairs (i, i+half) are
                # adjacent so the PE transpose reads one contiguous block
                # (walrus: matmul weight APs must have a single free dim).
                def slot(s):
                    return 2 * (s % half) + (s // half)

                # rounded copy of the state for matmul consumption
                xr_prev = xrp.tile([N, BSH], mdt, tag="xr", name="xr_init")
                nc.vector.tensor_copy(xr_prev[:], x0_c[:])
                # th_0 = tanh(C x_0 + by), fp32 matmul (one-time)
                psy0 = psyp.tile([N, BSH], F32, tag="psy", name="psy0")
                nc.tensor.matmul(psy0[:], CTf32[:], x0_c[:], start=True, stop=True)
                th_prev = thp.tile([N, BSH], mdt, tag="th", name="th_init")
                nc.scalar.activation(
                    th_prev[:], psy0[:], ACTF.Tanh, bias=by_c[:], scale=1.0
                )

                xb_prev = None
                for c in range(nchunks):
                    xb = xbufp.tile([N, tc_chunk * BSH], F32, tag="xb")
                    st = stagep.tile([128, half * N], F32, tag="st")
                    if c == 0:
                        nc.vector.tensor_copy(xb[:, 0:BSH], x0_c[:])
                    for s in range(tc_chunk):
                        t = c * tc_chunk + s
                        if t > 0:
                            if s > 0:
                                pxb, ps_ = xb, slot(s - 1)
                            else:
                                pxb, ps_ = xb_prev, slot(tc_chunk - 1)
                            xprev_f32 = pxb[:, ds(ps_ * BSH, BSH)]
                            # th-chain: y_t = P xr + Q th ; th_t = tanh(y+r)
                            psy = psyp.tile([N, BSH], F32, tag="psy")
                            nc.tensor.matmul(
                                psy[:], QTm[:], th_prev[:], start=True, stop=False
                            )
                            nc.tensor.matmul(
                                psy[:], PTm[:], xr_prev[:], start=False, stop=True
                            )
                            # x-chain: psx = 0.01A xr + 0.01B th
                            psx = psxp.tile([N, BSH], F32, tag="psx")
                            nc.tensor.matmul(
                                psx[:], BpTm[:], th_prev[:], start=True, stop=False
                            )
                            nc.tensor.matmul(
                                psx[:], A01Tm[:], xr_prev[:], start=False, stop=True
                            )
                            th_new = thp.tile([N, BSH], mdt, tag="th")
                            nc.scalar.activation(
                                th_new[:], psy[:], ACTF.Tanh, bias=r_c[:], scale=1.0
                            )
                            # chain-critical: rounded next state for the matmuls
                            xr_new = xrp.tile([N, BSH], mdt, tag="xr")
                            nc.vector.scalar_tensor_tensor(
                                xr_new[:], psx[:], bxp_c[:], xprev_f32,
                                op0=ALU.add, op1=ALU.add,
                            )
                            # exact fp32 state carry (also the output value)
                            nc.vector.scalar_tensor_tensor(
                                xb[:, ds(slot(s) * BSH, BSH)], psx[:], bxp_c[:],
                                xprev_f32, op0=ALU.add, op1=ALU.add,
                            )
                            xr_prev = xr_new
                            th_prev = th_new
                        if s >= half:
                            i = s - half
                            # transpose steps (i, i+half): adjacent slots
                            # (2i, 2i+1) -> one contiguous 128-col block.
                            # Two pair-transposes share one psum tile; a
                            # single ACT copy drains both (fewer ACT ops on
                            # the engine the chain-critical tanh runs on).
                            if i % 2 == 0:
                                pstr = pstrp.tile([128, 2 * N], F32, tag="pstr")
                            nc.tensor.transpose(
                                pstr[:, ds((i % 2) * N, N)],
                                xb[:, ds(2 * i * BSH, 2 * BSH)],
                                ident[:],
                            )
                            if i % 2 == 1 or s == tc_chunk - 1:
                                lo = (i - (i % 2)) * N
                                width = (i % 2 + 1) * N
                                nc.scalar.copy(
                                    st[:, ds(lo, width)], pstr[:, 0:width]
                                )
                    qn = max(half // 8, 1)
                    for h in range(2):
                        for q0 in range(0, half, qn):
                            t0 = c * tc_chunk + h * half + q0
                            dram_ap = out[:, t0:t0 + qn, :].rearrange(
                                "b i n -> b (i n)"
                            )
                            nc.sync.dma_start(
                                out=dram_ap,
                                in_=st[h * 64:(h + 1) * 64, ds(q0 * N, qn * N)],
                            )
                    xb_prev = xb

    nc.compile()
    return nc


_CACHED = {}


def _get_program(tmax=TMAX, tc_chunk=32, mdt=BF16):
    key = (tmax, tc_chunk, str(mdt))
    if key not in _CACHED:
        _CACHED[key] = build_program(tmax, tc_chunk, mdt)
    return _CACHED[key]


def make_in_maps(inputs, tmax=TMAX):
    X0 = np.ascontiguousarray(np.asarray(inputs["X0"], dtype=np.float32))
    base = {
        name: np.ascontiguousarray(np.asarray(inputs[name], dtype=np.float32))
        for name in PARAM_NAMES
    }
    base["bx"] = np.ascontiguousarray(
        np.asarray(inputs["bx"], dtype=np.float32).reshape(N, 1)
    )
    base["by"] = np.ascontiguousarray(
        np.asarray(inputs["by"], dtype=np.float32).reshape(N, 1)
    )
    in_maps = []
    for c in range(NCORES):
        m = dict(base)
        m["x0"] = np.ascontiguousarray(X0[c * BSH:(c + 1) * BSH].T)
        in_maps.append(m)
    return in_maps


def run_spmd(inputs, tmax=TMAX, tc_chunk=32, trace=False, tmpdir=None, mdt=BF16):
    nc = _get_program(tmax, tc_chunk, mdt)
    in_maps = make_in_maps(inputs, tmax)
    res = run_bass_kernel_spmd(
        nc, in_maps, list(range(NCORES)), trace=trace, tmpdir=tmpdir
    )
    outs = [res.results[c]["out"] for c in range(NCORES)]
    full = np.concatenate(outs, axis=0)
    return full, res


def kernel(**inputs):
    full, _ = run_spmd(inputs)
    return full



# revision 2
# speedup vs baseline: 1.3744x; 1.3744x over previous
"""LurieNet-k Trainium2 kernel (lag-2 restructured recurrence).

Computes, from the raw parametrization tensors, the matrices
  C = UC @ SC @ VC^T,  B = UB @ SB @ VB^T,
  A = 0.5*UA @ SA @ UA^T + 0.5*YA  (SA = -(alpha_upp*I + GA))
entirely on device (matrix exponentials of skew matrices via
scaling-and-squaring Taylor), then runs the 511-step recurrence
  u_t = tanh(C x_t + by);  x_{t+1} = x_t + 0.01*(A x_t + B u_t + bx)
on a (128, 64) state shard per NeuronCore (batch data-parallel over
the 8 cores).

The baseline (one tanh -> matmul -> tanh round trip per step) is
latency-bound at ~860ns/step: ACT's SBUF-write access latency (~370ns)
plus two semaphore hops plus the PE matmul put a ~820ns floor on the
serial u-loop. This version halves the serial depth with an EXACT
lag-2 expansion (one dropped term of norm ~1e-6/step):
  u_{t+1} = tanh(P2 x_{t-1} + Qc u_{t-1} + r2)     P2 = C*At^2
                                                   Qc = C*(At+I)*G
  x_{t+1} = x_t + (A2 x_{t-1} + G2 u_t + c2)       A2 = 0.01*A*At
                                                   G2 = At*G
with At = I + 0.01A, G = 0.01B, c = 0.01bx, r2 = C*(At+I)*c + by,
c2 = At*c. The u-chain self-dependence now spans two steps (even/odd
chains interleave), so the per-step period is bounded by PE matmul
throughput (~4 LDW+MM pairs) instead of the ACT round-trip latency.
The dropped term Q*(u_t - u_{t-1}) has norm ~3e-9 per step (||C*G||
~ 3e-4); measured scheme error vs exact fp64 is 6e-7.

Split-carry precision: the fp32 state carry lives on DVE (one
fused add per step); a bf16 rounded copy for the matmuls is made on
the otherwise-idle GPSIMD engine. Output is written bf16 time-major
[n, t, b] straight from the state buffer (no on-device transpose:
the PE transposes, ACT staging copies, and per-4-step DMA issues of
the baseline are all gone) and transposed to (b, t, n) fp32 on the
host during unsharding; the t=0 plane is restored exactly from X0.
Measured end-to-end rel err ~2.6e-3 (budget 2e-2).
"""

import sys

for _p in ("/opt/trn_rl_repo",):
    if _p not in sys.path:
        sys.path.insert(0, _p)

import numpy as np

import concourse.bass as bass
import concourse.mybir as mybir
import concourse.tile as tile
from concourse import bacc
from concourse import bass_isa
from concourse.bass import ds
from concourse.bass_utils import run_bass_kernel_spmd
from concourse.masks import make_identity, make_upper_triangular

F32 = mybir.dt.float32
F32R = mybir.dt.float32r
BF16 = mybir.dt.bfloat16
ALU = mybir.AluOpType
ACTF = mybir.ActivationFunctionType
AXIS = mybir.AxisListType

N = 128          # state dim
TMAX = 512       # time steps (including t=0)
BS = 512         # global batch
NCORES = 8
BSH = BS // NCORES   # 64 batch columns per core
STEP = 0.01
KTOP = 4

EXPM_SCAL = 3    # expm scaling: X = S / 2**EXPM_SCAL, then 3 squarings
EXPM_TERMS = 4   # Taylor terms in the Horner evaluation

PARAM_NAMES = [
    "ZA_Y", "ZA_U", "ZA_G", "ZB_U", "ZB_V", "ZB_S", "ZC_U", "ZC_V", "ZC_S",
]


def build_program(tmax=TMAX, tc_chunk=64, mdt=BF16):
    """Build the single-NeuronCore Bass program (run SPMD on all 8 cores).

    tc_chunk: output staging chunk (time steps per output DMA).
    """
    assert tmax % tc_chunk == 0
    nchunks = tmax // tc_chunk

    nc = bacc.Bacc(
        "TRN2",
        target_bir_lowering=False,
        debug=False,
        enable_asserts=False,
        num_devices=NCORES,
    )

    x0 = nc.dram_tensor("x0", [N, BSH], F32, kind="ExternalInput")
    zs = {
        name: nc.dram_tensor(name, [N, N], F32, kind="ExternalInput")
        for name in PARAM_NAMES
    }
    bx_d = nc.dram_tensor("bx", [N, 1], F32, kind="ExternalInput")
    by_d = nc.dram_tensor("by", [N, 1], F32, kind="ExternalInput")
    # time-major bf16 output: out[n, t*BSH + b]; host transposes to (b,t,n)
    out = nc.dram_tensor("out", [N, tmax * BSH], BF16, kind="ExternalOutput")

    with tile.TileContext(nc) as tc:
        with tc.tile_pool(name="const", bufs=1) as constp:
            ident = constp.tile([N, N], F32, tag="ident")
            make_identity(nc, ident[:])
            masku = constp.tile([N, N], F32, tag="masku")
            make_upper_triangular(nc, masku[:], val=1.0, diag=False)
            ident_r32 = constp.tile([N, N], F32R, tag="ident_r32")
            nc.vector.tensor_copy(ident_r32[:], ident[:])

            by_c = constp.tile([N, 1], F32, tag="by")
            nc.sync.dma_start(out=by_c[:], in_=by_d[:])
            bx_c = constp.tile([N, 1], F32, tag="bxraw")
            nc.sync.dma_start(out=bx_c[:], in_=bx_d[:])
            bxp_c = constp.tile([N, 1], F32, tag="bxp")
            nc.vector.tensor_scalar_mul(bxp_c[:], bx_c[:], STEP)
            x0_c = constp.tile([N, BSH], F32, tag="x0c")
            nc.sync.dma_start(out=x0_c[:], in_=x0[:])

            # ------- setup phase: expm's + weight assembly -------
            # Runtime weights (bf16, transposed for use as matmul lhsT):
            P2Tm = constp.tile([N, N], mdt, tag="P2Tm")    # (C At^2)^T
            QcTm = constp.tile([N, N], mdt, tag="QcTm")    # (C (At+I) G)^T
            A2Tm = constp.tile([N, N], mdt, tag="A2Tm")    # (0.01 A At)^T
            G2Tm = constp.tile([N, N], mdt, tag="G2Tm")    # (At G)^T
            A01Tm = constp.tile([N, N], mdt, tag="A01Tm")  # (0.01 A)^T (boot)
            BpTm = constp.tile([N, N], mdt, tag="BpTm")    # (0.01 B)^T (boot)
            CTf32 = constp.tile([N, N], F32, tag="CTf32")  # C^T fp32 (boot)
            r2_c = constp.tile([N, 1], F32, tag="r2c")     # C(At+I)c + by
            c2_c = constp.tile([N, 1], F32, tag="c2c")     # At c

            with (
                tc.tile_pool(name="zbuf", bufs=1) as zp,
                tc.tile_pool(name="work", bufs=2) as wp,
                tc.tile_pool(name="eres", bufs=1) as ep,
                tc.tile_pool(name="small", bufs=1) as sp,
                tc.tile_pool(name="pss", bufs=4, space="PSUM") as psp,
            ):
                zt = {}
                for name in PARAM_NAMES:
                    zt[name] = zp.tile([N, N], F32, tag=name, name=f"z_{name}")
                    nc.sync.dma_start(out=zt[name][:], in_=zs[name][:])

                def expm_batch(specs):
                    """Interleaved expm(skew(Z))^T for all matrices at once.

                    Maintains the (T, T^T) pair through Horner + squaring so
                    no PE transposes are needed: with negX = X^T = -X,
                      X @ T     = matmul(lhsT=negX, rhs=T)
                      T^T @ X^T = matmul(lhsT=T,    rhs=negX)
                    The five chains are advanced stage-by-stage so PE/DVE/ACT
                    work from different chains overlaps (a single sequential
                    chain is latency-bound on the MM->STT->MM dependency).
                    """
                    scal = 1.0 / (2.0 ** EXPM_SCAL)
                    negx = {}
                    t_cur = {}
                    tt_cur = {}
                    for z_tile, tag in specs:
                        us = wp.tile([N, N], F32R, tag="us_r", name=f"us_{tag}")
                        nc.vector.scalar_tensor_tensor(
                            us[:], z_tile[:], scal, masku[:],
                            op0=ALU.mult, op1=ALU.mult,
                        )
                        pst = psp.tile([N, N], F32R, tag="ps", bufs=8,
                                       name=f"pst_{tag}")
                        nc.tensor.transpose(pst[:], us[:], ident_r32[:])
                        nx = wp.tile([N, N], F32R, tag=f"negx_{tag}", bufs=1,
                                     name=f"negx_{tag}")
                        nc.vector.scalar_tensor_tensor(
                            nx[:], pst[:], 1.0, us[:],
                            op0=ALU.mult, op1=ALU.subtract,
                        )
                        negx[tag] = nx
                        t_cur[tag] = ident_r32
                        tt_cur[tag] = ident_r32
                    # Horner advances only T (the T^T pair is rebuilt by a
                    # PE transpose afterwards -- halves the vector-op load,
                    # which bounds this phase).
                    for j in range(EXPM_TERMS, 0, -1):
                        for _, tag in specs:
                            psa = psp.tile([N, N], F32, tag="ps", bufs=8)
                            nc.tensor.matmul(
                                psa[:], negx[tag][:], t_cur[tag][:],
                                start=True, stop=True,
                            )
                            t_new = wp.tile([N, N], F32R, tag=f"T_{tag}",
                                            bufs=2, name=f"T_{tag}")
                            nc.vector.scalar_tensor_tensor(
                                t_new[:], psa[:], 1.0 / j, ident_r32[:],
                                op0=ALU.mult, op1=ALU.add,
                            )
                            t_cur[tag] = t_new
                    for _, tag in specs:
                        pst = psp.tile([N, N], F32R, tag="ps", bufs=8,
                                       name=f"ptt_{tag}")
                        nc.tensor.transpose(pst[:], t_cur[tag][:], ident_r32[:])
                        tt_new = wp.tile([N, N], F32R, tag=f"TT_{tag}",
                                         bufs=2, name=f"TT_{tag}")
                        nc.scalar.copy(tt_new[:], pst[:])
                        tt_cur[tag] = tt_new
                    for _ in range(EXPM_SCAL):
                        for _, tag in specs:
                            psa = psp.tile([N, N], F32, tag="ps", bufs=8)
                            psb = psp.tile([N, N], F32, tag="ps", bufs=8)
                            nc.tensor.matmul(
                                psa[:], tt_cur[tag][:], t_cur[tag][:],
                                start=True, stop=True,
                            )
                            nc.tensor.matmul(
                                psb[:], t_cur[tag][:], tt_cur[tag][:],
                                start=True, stop=True,
                            )
                            t_new = wp.tile([N, N], F32R, tag=f"T_{tag}",
                                            bufs=2, name=f"T_{tag}")
                            tt_new = wp.tile([N, N], F32R, tag=f"TT_{tag}",
                                             bufs=2, name=f"TT_{tag}")
                            nc.vector.tensor_copy(t_new[:], psa[:])
                            nc.scalar.copy(tt_new[:], psb[:])
                            t_cur[tag], tt_cur[tag] = t_new, tt_new
                    return tt_cur

                eres = expm_batch([
                    (zt["ZC_U"], "UCT"), (zt["ZC_V"], "VCT"),
                    (zt["ZB_U"], "UBT"), (zt["ZB_V"], "VBT"),
                    (zt["ZA_U"], "UAT"),
                ])
                uct, vct = eres["UCT"], eres["VCT"]
                ubt, vbt = eres["UBT"], eres["VBT"]
                uat = eres["UAT"]

                def absdiag_col(z_tile, tag):
                    tmp = wp.tile([N, N], F32, tag="us")
                    nc.vector.tensor_mul(tmp[:], z_tile[:], ident[:])
                    col = sp.tile([N, 1], F32, tag=tag, name=f"col_{tag}")
                    nc.vector.tensor_reduce(
                        col[:], tmp[:], AXIS.X, ALU.add, apply_absolute_value=True
                    )
                    return col

                dc_col = absdiag_col(zt["ZC_S"], "dc")   # |diag(ZC_S)|
                db_col = absdiag_col(zt["ZB_S"], "db")   # |diag(ZB_S)|
                ga_col = absdiag_col(zt["ZA_G"], "ga")   # |diag(ZA_G)|

                # top-4: alpha = sqrt(sum_i (b_i c_i)^2), b/c sorted desc.
                bwork = sp.tile([N, 1], F32, tag="bwork")
                cwork = sp.tile([N, 1], F32, tag="cwork")
                nc.vector.tensor_copy(bwork[:], db_col[:])
                nc.vector.tensor_copy(cwork[:], dc_col[:])
                acc = sp.tile([N, 1], F32, tag="acc")
                nc.vector.memset(acc[:], 0.0)
                bmax = sp.tile([N, 1], F32, tag="bmax")
                cmax = sp.tile([N, 1], F32, tag="cmax")
                prod = sp.tile([N, 1], F32, tag="prod")
                gmask = sp.tile([N, 1], F32, tag="gmask")
                tdrop = sp.tile([N, 1], F32, tag="tdrop")
                for i in range(KTOP):
                    nc.gpsimd.partition_all_reduce(
                        bmax[:], bwork[:], N, bass_isa.ReduceOp.max
                    )
                    nc.gpsimd.partition_all_reduce(
                        cmax[:], cwork[:], N, bass_isa.ReduceOp.max
                    )
                    nc.vector.tensor_mul(prod[:], bmax[:], cmax[:])
                    nc.vector.tensor_mul(prod[:], prod[:], prod[:])
                    nc.vector.tensor_add(acc[:], acc[:], prod[:])
                    if i < KTOP - 1:
                        # zero out the extracted max (values all > 0)
                        nc.vector.tensor_single_scalar(
                            gmask[:], bwork[:], bmax[:], ALU.is_ge
                        )
                        nc.vector.tensor_mul(tdrop[:], bwork[:], gmask[:])
                        nc.vector.tensor_sub(bwork[:], bwork[:], tdrop[:])
                        nc.vector.tensor_single_scalar(
                            gmask[:], cwork[:], cmax[:], ALU.is_ge
                        )
                        nc.vector.tensor_mul(tdrop[:], cwork[:], gmask[:])
                        nc.vector.tensor_sub(cwork[:], cwork[:], tdrop[:])
                alpha = sp.tile([N, 1], F32, tag="alpha")
                nc.scalar.activation(alpha[:], acc[:], ACTF.Sqrt)

                # sa05 = -0.5*(alpha + gA)  (per-partition row scale of UA^T)
                sa05 = sp.tile([N, 1], F32, tag="sa05")
                nc.vector.tensor_scalar(
                    sa05[:], ga_col[:], alpha[:], -0.5, op0=ALU.add, op1=ALU.mult
                )
                sb01 = sp.tile([N, 1], F32, tag="sb01")
                nc.vector.tensor_scalar_mul(sb01[:], db_col[:], STEP)

                # C^T = VC @ (SC @ UC^T)
                p1 = wp.tile([N, N], F32R, tag="us_r", name="p1")
                nc.vector.tensor_scalar_mul(p1[:], uct[:], dc_col[:])
                psa = psp.tile([N, N], F32, tag="ps", bufs=8)
                nc.tensor.matmul(psa[:], vct[:], p1[:], start=True, stop=True)
                nc.vector.tensor_copy(CTf32[:], psa[:])

                # (0.01 B)^T = VB @ (0.01 SB @ UB^T)
                p2 = wp.tile([N, N], F32R, tag="us_r", name="p2")
                nc.vector.tensor_scalar_mul(p2[:], ubt[:], sb01[:])
                psb = psp.tile([N, N], F32, tag="ps", bufs=8)
                nc.tensor.matmul(psb[:], vbt[:], p2[:], start=True, stop=True)
                nc.vector.tensor_copy(BpTm[:], psb[:])
                BpTf32 = ep.tile([N, N], F32, tag="BpTf32")
                nc.scalar.copy(BpTf32[:], psb[:])
                # untransposed 0.01 B = UB @ (0.01 SB @ VB^T)
                p2b = wp.tile([N, N], F32R, tag="us_r", name="p2b")
                nc.vector.tensor_scalar_mul(p2b[:], vbt[:], sb01[:])
                psb2 = psp.tile([N, N], F32, tag="ps", bufs=8)
                nc.tensor.matmul(psb2[:], ubt[:], p2b[:], start=True, stop=True)
                bp_un = ep.tile([N, N], F32, tag="Bpun")
                nc.vector.tensor_copy(bp_un[:], psb2[:])

                # M = UA @ (sa05 * UA^T) = 0.5*UA SA UA^T (symmetric)
                p3 = wp.tile([N, N], F32R, tag="us_r", name="p3")
                nc.vector.tensor_scalar_mul(p3[:], uat[:], sa05[:])
                psm = psp.tile([N, N], F32, tag="ps", bufs=8)
                nc.tensor.matmul(psm[:], uat[:], p3[:], start=True, stop=True)
                # YA = Uy - Uy^T; q2 = -0.005*YA
                uy = wp.tile([N, N], F32, tag="us")
                nc.vector.tensor_mul(uy[:], zt["ZA_Y"][:], masku[:])
                pst2 = psp.tile([N, N], F32, tag="ps", bufs=8)
                nc.tensor.transpose(pst2[:], uy[:], ident[:])
                nc.vector.tensor_scalar_mul(uy[:], uy[:], 0.5 * STEP)
                q2 = wp.tile([N, N], F32, tag="T")
                nc.vector.scalar_tensor_tensor(
                    q2[:], pst2[:], 0.5 * STEP, uy[:], op0=ALU.mult, op1=ALU.subtract
                )
                # (0.01 A)^T = 0.01*M + q2 ; untransposed 0.01 A = 0.01*M - q2
                A01Tf32 = ep.tile([N, N], F32, tag="A01Tf32")
                nc.vector.scalar_tensor_tensor(
                    A01Tf32[:], psm[:], STEP, q2[:], op0=ALU.mult, op1=ALU.add
                )
                nc.vector.tensor_copy(A01Tm[:], A01Tf32[:])
                a01_un = ep.tile([N, N], F32, tag="A01un")
                nc.vector.scalar_tensor_tensor(
                    a01_un[:], psm[:], STEP, q2[:], op0=ALU.mult, op1=ALU.subtract
                )

                # P^T = C^T + (0.01 A)^T C^T   (fp32 intermediate)
                PTf32 = ep.tile([N, N], F32, tag="PTf32")
                psw = psp.tile([N, N], F32, tag="ps", bufs=8)
                nc.tensor.matmul(psw[:], a01_un[:], CTf32[:], start=True, stop=True)
                nc.vector.scalar_tensor_tensor(
                    PTf32[:], psw[:], 1.0, CTf32[:], op0=ALU.mult, op1=ALU.add
                )

                # P2^T = At^T P^T = P^T + (0.01A)^T P^T
                psp2 = psp.tile([N, N], F32, tag="ps", bufs=8)
                nc.tensor.matmul(psp2[:], a01_un[:], PTf32[:], start=True, stop=True)
                nc.vector.scalar_tensor_tensor(
                    P2Tm[:], psp2[:], 1.0, PTf32[:], op0=ALU.mult, op1=ALU.add
                )

                # Qc^T = G^T (At^T + I) C^T = G^T (P^T + C^T)
                scp = wp.tile([N, N], F32, tag="us")
                nc.vector.tensor_add(scp[:], CTf32[:], PTf32[:])
                psq = psp.tile([N, N], F32, tag="ps", bufs=8)
                nc.tensor.matmul(psq[:], bp_un[:], scp[:], start=True, stop=True)
                nc.vector.tensor_copy(QcTm[:], psq[:])

                # A2^T = (0.01A)^T + (0.01A)^T (0.01A)^T
                psa2 = psp.tile([N, N], F32, tag="ps", bufs=8)
                nc.tensor.matmul(psa2[:], a01_un[:], A01Tf32[:], start=True, stop=True)
                nc.vector.scalar_tensor_tensor(
                    A2Tm[:], psa2[:], 1.0, A01Tf32[:], op0=ALU.mult, op1=ALU.add
                )

                # G2^T = (At G)^T = G^T + G^T (0.01A)^T
                psg2 = psp.tile([N, N], F32, tag="ps", bufs=8)
                nc.tensor.matmul(psg2[:], bp_un[:], A01Tf32[:], start=True, stop=True)
                nc.vector.scalar_tensor_tensor(
                    G2Tm[:], psg2[:], 1.0, BpTf32[:], op0=ALU.mult, op1=ALU.add
                )

                # r2 = C (At + I) c + by ; c2 = At c   (c = 0.01 bx)
                psr = psp.tile([N, 1], F32, tag="ps", bufs=8, name="psr")
                nc.tensor.matmul(psr[:], A01Tf32[:], bxp_c[:], start=True, stop=True)
                v1 = sp.tile([N, 1], F32, tag="v1")
                nc.vector.scalar_tensor_tensor(
                    v1[:], bxp_c[:], 2.0, psr[:], op0=ALU.mult, op1=ALU.add
                )
                nc.vector.scalar_tensor_tensor(
                    c2_c[:], bxp_c[:], 1.0, psr[:], op0=ALU.mult, op1=ALU.add
                )
                psr2 = psp.tile([N, 1], F32, tag="ps", bufs=8, name="psr2")
                nc.tensor.matmul(psr2[:], CTf32[:], v1[:], start=True, stop=True)
                nc.vector.scalar_tensor_tensor(
                    r2_c[:], psr2[:], 1.0, by_c[:], op0=ALU.mult, op1=ALU.add
                )

            # ------- recurrence (lag-2, split-carry) -------
            with (
                tc.tile_pool(name="xrb", bufs=2) as xrbp,
                tc.tile_pool(name="xb", bufs=4) as xbp,
                tc.tile_pool(name="th", bufs=4) as thp,
                tc.tile_pool(name="psy", bufs=4, space="PSUM") as psyp,
                tc.tile_pool(name="psx", bufs=4, space="PSUM") as psxp,
            ):
                CH = tc_chunk

                # chunk 0 buffer; xr_0 = bf16(x0)
                xrb = xrbp.tile([N, CH * BSH], mdt, tag="xrb")
                nc.gpsimd.tensor_copy(xrb[:, 0:BSH], x0_c[:])
                xr_loc = {0: (xrb, 0)}

                # u_0 = tanh(C x_0 + by)  (one-time fp32 matmul)
                psy0 = psyp.tile([N, BSH], F32, tag="psy", name="psy0")
                nc.tensor.matmul(psy0[:], CTf32[:], x0_c[:], start=True, stop=True)
                u0 = thp.tile([N, BSH], mdt, tag="th", name="u0")
                nc.scalar.activation(
                    u0[:], psy0[:], ACTF.Tanh, bias=by_c[:], scale=1.0
                )
                # x_1 = x_0 + (0.01A x_0 + 0.01B u_0 + 0.01bx)
                psx0 = psxp.tile([N, BSH], F32, tag="psx", name="psx0")
                nc.tensor.matmul(
                    psx0[:], A01Tm[:], xrb[:, 0:BSH], start=True, stop=False
                )
                nc.tensor.matmul(psx0[:], BpTm[:], u0[:], start=False, stop=True)
                xb1 = xbp.tile([N, BSH], F32, tag="xb", name="xb1")
                nc.vector.scalar_tensor_tensor(
                    xb1[:], psx0[:], bxp_c[:], x0_c[:], op0=ALU.add, op1=ALU.add
                )
                nc.gpsimd.tensor_copy(xrb[:, ds(BSH, BSH)], xb1[:])
                xr_loc[1] = (xrb, 1)
                # u_1 = tanh(C x_1 + by)  (one-time fp32 matmul)
                psy1 = psyp.tile([N, BSH], F32, tag="psy", name="psy1")
                nc.tensor.matmul(psy1[:], CTf32[:], xb1[:], start=True, stop=True)
                u1 = thp.tile([N, BSH], mdt, tag="th", name="u1")
                nc.scalar.activation(
                    u1[:], psy1[:], ACTF.Tanh, bias=by_c[:], scale=1.0
                )

                u_hist = {0: u0, 1: u1}
                xb_prev = xb1  # x_{t-1} fp32 carry

                for t in range(2, tmax):
                    s = t % CH
                    if s == 0:
                        xrb = xrbp.tile([N, CH * BSH], mdt, tag="xrb")
                    xr2_buf, xr2_slot = xr_loc[t - 2]
                    xr2 = xr2_buf[:, ds(xr2_slot * BSH, BSH)]

                    # psy_t = P2 xr_{t-2} + Qc u_{t-2}
                    psy = psyp.tile([N, BSH], F32, tag="psy")
                    nc.tensor.matmul(psy[:], P2Tm[:], xr2, start=True, stop=False)
                    nc.tensor.matmul(
                        psy[:], QcTm[:], u_hist[t - 2][:], start=False, stop=True
                    )
                    # psx_t = A2 xr_{t-2} + G2 u_{t-1}
                    psx = psxp.tile([N, BSH], F32, tag="psx")
                    nc.tensor.matmul(psx[:], A2Tm[:], xr2, start=True, stop=False)
                    nc.tensor.matmul(
                        psx[:], G2Tm[:], u_hist[t - 1][:], start=False, stop=True
                    )
                    # u_t = tanh(psy + r2)
                    u_t = thp.tile([N, BSH], mdt, tag="th")
                    nc.scalar.activation(
                        u_t[:], psy[:], ACTF.Tanh, bias=r2_c[:], scale=1.0
                    )
                    # x_t = x_{t-1} + (psx + c2)  (fp32 carry on DVE)
                    xb_t = xbp.tile([N, BSH], F32, tag="xb")
                    nc.vector.scalar_tensor_tensor(
                        xb_t[:], psx[:], c2_c[:], xb_prev[:],
                        op0=ALU.add, op1=ALU.add,
                    )
                    # xr_t = bf16(x_t) into the chunk buffer (GPSIMD)
                    nc.gpsimd.tensor_copy(xrb[:, ds(s * BSH, BSH)], xb_t[:])

                    u_hist[t] = u_t
                    u_hist.pop(t - 2, None)
                    xr_loc[t] = (xrb, s)
                    xr_loc.pop(t - 3, None)
                    xb_prev = xb_t

                    if s == CH - 1:
                        c = t // CH
                        nc.sync.dma_start(
                            out=out[:, ds(c * CH * BSH, CH * BSH)],
                            in_=xrb[:, 0:CH * BSH],
                        )

    nc.compile()
    return nc


_CACHED = {}


def _get_program(tmax=TMAX, tc_chunk=64, mdt=BF16):
    key = (tmax, tc_chunk, str(mdt))
    if key not in _CACHED:
        _CACHED[key] = build_program(tmax, tc_chunk, mdt)
    return _CACHED[key]


def make_in_maps(inputs, tmax=TMAX):
    X0 = np.ascontiguousarray(np.asarray(inputs["X0"], dtype=np.float32))
    base = {
        name: np.ascontiguousarray(np.asarray(inputs[name], dtype=np.float32))
        for name in PARAM_NAMES
    }
    base["bx"] = np.ascontiguousarray(
        np.asarray(inputs["bx"], dtype=np.float32).reshape(N, 1)
    )
    base["by"] = np.ascontiguousarray(
        np.asarray(inputs["by"], dtype=np.float32).reshape(N, 1)
    )
    in_maps = []
    for c in range(NCORES):
        m = dict(base)
        m["x0"] = np.ascontiguousarray(X0[c * BSH:(c + 1) * BSH].T)
        in_maps.append(m)
    return in_maps


def run_spmd(inputs, tmax=TMAX, tc_chunk=64, trace=False, tmpdir=None, mdt=BF16):
    nc = _get_program(tmax, tc_chunk, mdt)
    in_maps = make_in_maps(inputs, tmax)
    res = run_bass_kernel_spmd(
        nc, in_maps, list(range(NCORES)), trace=trace, tmpdir=tmpdir
    )
    X0 = np.asarray(inputs["X0"], dtype=np.float32)
    outs = []
    for c in range(NCORES):
        o = np.asarray(res.results[c]["out"])        # [N, tmax*BSH] bf16
        o = o.reshape(N, tmax, BSH).transpose(2, 1, 0).astype(np.float32)
        outs.append(o)                               # (BSH, tmax, N)
    full = np.concatenate(outs, axis=0)              # (BS, tmax, N)
    full[:, 0, :] = X0                               # exact t=0 plane
    return full, res


def kernel(**inputs):
    full, _ = run_spmd(inputs)
    return full


# revision 8
# speedup vs baseline: 1.9621x; 1.4277x over previous
"""LurieNet-k Trainium2 kernel (lag-4 paired recurrence, fp16 operands).

Computes, from the raw parametrization tensors, the matrices
  C = UC @ SC @ VC^T,  B = UB @ SB @ VB^T,
  A = 0.5*UA @ SA @ UA^T + 0.5*YA  (SA = -(alpha_upp*I + GA))
entirely on device (matrix exponentials of skew matrices via
scaling-and-squaring Taylor), then runs the 511-step recurrence
  u_t = tanh(C x_t + by);  x_{t+1} = x_t + 0.01*(A x_t + B u_t + bx)
on a (128, 64) state shard per NeuronCore (batch data-parallel over
the 8 cores).

Structure: the naive step is a serial tanh->matmul->tanh round trip
(~860ns on TRN2: ACT access latency + 2 sem hops + PE). Because the
tanh self-coupling Q = 0.01*C*B has tiny norm (~3e-4), the recurrence
is re-expanded to an (almost) exact LAG-4 form: every quantity at
step t is computed from state/tanh values at steps t-4/t-3 (and t-8
/t-7 for a first-order staleness extrapolation of the x-chain's u
terms, which kills the dominant scheme error: measured 1.2e-2 plain
-> 3.2e-4 extrapolated, fp16). All matmul inputs are then >= 2
pair-iterations old, so nothing serializes, and steps are processed
in PAIRS:
  - one 128-wide matmul per weight per pair (halves LDWEIGHTS, the
    PE throughput limit)
  - one 128-wide tanh per pair (halves ACT's per-instruction access
    latency tax)
  - PSUM evacuation split across ACT (Copy w/ bias for delta_t),
    DVE (pair-sum + the two fp16 state writes), and GPSIMD (fp32
    pair carry, SBUF-only since GPSIMD cannot read PSUM).
Weights/states/tanh values are fp16 (same PE speed as bf16, 8x finer
rounding); the fp32 carry keeps the state exact (all partial sums
fp32). Output is written fp16 time-major [n, t, b] straight from the
state buffer (no on-device transpose) and transposed to (b, t, n)
fp32 on the host during unsharding; the t=0 plane is restored
exactly from X0. Measured end-to-end rel err ~4e-4 (budget 2e-2).
"""

import sys

for _p in ("/opt/trn_rl_repo",):
    if _p not in sys.path:
        sys.path.insert(0, _p)

import numpy as np

import concourse.bass as bass
import concourse.mybir as mybir
import concourse.tile as tile
from concourse import bacc
from concourse import bass_isa
from concourse.bass import ds
from concourse.bass_utils import run_bass_kernel_spmd
from concourse.masks import make_identity, make_upper_triangular

F32 = mybir.dt.float32
F32R = mybir.dt.float32r
FP16 = mybir.dt.float16
ALU = mybir.AluOpType
ACTF = mybir.ActivationFunctionType
AXIS = mybir.AxisListType

N = 128          # state dim
TMAX = 512       # time steps (including t=0)
BS = 512         # global batch
NCORES = 8
BSH = BS // NCORES   # 64 batch columns per core
STEP = 0.01
KTOP = 4

EXPM_SCAL = 3    # expm scaling: X = S / 2**EXPM_SCAL, then 3 squarings
EXPM_TERMS = 4   # Taylor terms in the Horner evaluation

PARAM_NAMES = [
    "ZA_Y", "ZA_U", "ZA_G", "ZB_U", "ZB_V", "ZB_S", "ZC_U", "ZC_V", "ZC_S",
]


def build_program(tmax=TMAX, tc_chunk=64, mdt=FP16):
    """Build the single-NeuronCore Bass program (run SPMD on all 8 cores)."""
    assert tmax % tc_chunk == 0 and tc_chunk % 2 == 0
    nc = bacc.Bacc(
        "TRN2",
        target_bir_lowering=False,
        debug=False,
        enable_asserts=False,
        num_devices=NCORES,
    )

    x0 = nc.dram_tensor("x0", [N, BSH], F32, kind="ExternalInput")
    zs = {
        name: nc.dram_tensor(name, [N, N], F32, kind="ExternalInput")
        for name in PARAM_NAMES
    }
    bx_d = nc.dram_tensor("bx", [N, 1], F32, kind="ExternalInput")
    by_d = nc.dram_tensor("by", [N, 1], F32, kind="ExternalInput")
    # time-major fp16 output: out[n, t*BSH + b]; host transposes to (b,t,n)
    out = nc.dram_tensor("out", [N, tmax * BSH], FP16, kind="ExternalOutput")

    with tile.TileContext(nc) as tc:
        with tc.tile_pool(name="const", bufs=1) as constp:
            ident = constp.tile([N, N], F32, tag="ident")
            make_identity(nc, ident[:])
            masku = constp.tile([N, N], F32, tag="masku")
            make_upper_triangular(nc, masku[:], val=1.0, diag=False)
            ident_r32 = constp.tile([N, N], F32R, tag="ident_r32")
            nc.vector.tensor_copy(ident_r32[:], ident[:])

            by_c = constp.tile([N, 1], F32, tag="by")
            nc.sync.dma_start(out=by_c[:], in_=by_d[:])
            bx_c = constp.tile([N, 1], F32, tag="bxraw")
            nc.sync.dma_start(out=bx_c[:], in_=bx_d[:])
            bxp_c = constp.tile([N, 1], F32, tag="bxp")
            nc.vector.tensor_scalar_mul(bxp_c[:], bx_c[:], STEP)
            x0_c = constp.tile([N, BSH], F32, tag="x0c")
            nc.sync.dma_start(out=x0_c[:], in_=x0[:])

            # ------- runtime weights (transposed, fp16) -------
            P4Tm = constp.tile([N, N], mdt, tag="P4Tm")    # (C At^4)^T
            Q4Tm = constp.tile([N, N], mdt, tag="Q4Tm")    # (C S3 G)^T
            A4Tm = constp.tile([N, N], mdt, tag="A4Tm")    # (0.01A At^3)^T
            GATm = constp.tile([N, N], mdt, tag="GATm")    # extrap u_{t-4} w
            GBTm = constp.tile([N, N], mdt, tag="GBTm")    # extrap u_{t-8} w
            G4Tm = constp.tile([N, N], mdt, tag="G4Tm")    # plain (boot pairs)
            A01Tm = constp.tile([N, N], mdt, tag="A01Tm")  # (0.01 A)^T (boot)
            BpTm = constp.tile([N, N], mdt, tag="BpTm")    # (0.01 B)^T (boot)
            CTf32 = constp.tile([N, N], F32, tag="CTf32")  # C^T fp32 (boot)
            r4_c = constp.tile([N, 1], F32, tag="r4c")     # C S3 c + by
            cc_c = constp.tile([N, 1], F32, tag="ccc")     # delta const

            with (
                tc.tile_pool(name="zbuf", bufs=1) as zp,
                tc.tile_pool(name="work", bufs=2) as wp,
                tc.tile_pool(name="eres", bufs=1) as ep,
                tc.tile_pool(name="small", bufs=1) as sp,
                tc.tile_pool(name="pss", bufs=4, space="PSUM") as psp,
            ):
                zt = {}
                for name in PARAM_NAMES:
                    zt[name] = zp.tile([N, N], F32, tag=name, name=f"z_{name}")
                    nc.sync.dma_start(out=zt[name][:], in_=zs[name][:])

                def expm_batch(specs):
                    """Interleaved expm(skew(Z))^T for all matrices at once.

                    Maintains the (T, T^T) pair through Horner + squaring so
                    no PE transposes are needed: with negX = X^T = -X,
                      X @ T     = matmul(lhsT=negX, rhs=T)
                      T^T @ X^T = matmul(lhsT=T,    rhs=negX)
                    """
                    scal = 1.0 / (2.0 ** EXPM_SCAL)
                    negx = {}
                    t_cur = {}
                    tt_cur = {}
                    for z_tile, tag in specs:
                        us = wp.tile([N, N], F32R, tag="us_r", name=f"us_{tag}")
                        nc.vector.scalar_tensor_tensor(
                            us[:], z_tile[:], scal, masku[:],
                            op0=ALU.mult, op1=ALU.mult,
                        )
                        pst = psp.tile([N, N], F32R, tag="ps", bufs=8,
                                       name=f"pst_{tag}")
                        nc.tensor.transpose(pst[:], us[:], ident_r32[:])
                        nx = wp.tile([N, N], F32R, tag=f"negx_{tag}", bufs=1,
                                     name=f"negx_{tag}")
                        nc.vector.scalar_tensor_tensor(
                            nx[:], pst[:], 1.0, us[:],
                            op0=ALU.mult, op1=ALU.subtract,
                        )
                        negx[tag] = nx
                        t_cur[tag] = ident_r32
                        tt_cur[tag] = ident_r32
                    for j in range(EXPM_TERMS, 0, -1):
                        for _, tag in specs:
                            psa = psp.tile([N, N], F32, tag="ps", bufs=8)
                            nc.tensor.matmul(
                                psa[:], negx[tag][:], t_cur[tag][:],
                                start=True, stop=True,
                            )
                            t_new = wp.tile([N, N], F32R, tag=f"T_{tag}",
                                            bufs=2, name=f"T_{tag}")
                            nc.vector.scalar_tensor_tensor(
                                t_new[:], psa[:], 1.0 / j, ident_r32[:],
                                op0=ALU.mult, op1=ALU.add,
                            )
                            t_cur[tag] = t_new
                    for _, tag in specs:
                        pst = psp.tile([N, N], F32R, tag="ps", bufs=8,
                                       name=f"ptt_{tag}")
                        nc.tensor.transpose(pst[:], t_cur[tag][:], ident_r32[:])
                        tt_new = wp.tile([N, N], F32R, tag=f"TT_{tag}",
                                         bufs=2, name=f"TT_{tag}")
                        nc.scalar.copy(tt_new[:], pst[:])
                        tt_cur[tag] = tt_new
                    for _ in range(EXPM_SCAL):
                        for _, tag in specs:
                            psa = psp.tile([N, N], F32, tag="ps", bufs=8)
                            psb = psp.tile([N, N], F32, tag="ps", bufs=8)
                            nc.tensor.matmul(
                                psa[:], tt_cur[tag][:], t_cur[tag][:],
                                start=True, stop=True,
                            )
                            nc.tensor.matmul(
                                psb[:], t_cur[tag][:], tt_cur[tag][:],
                                start=True, stop=True,
                            )
                            t_new = wp.tile([N, N], F32R, tag=f"T_{tag}",
                                            bufs=2, name=f"T_{tag}")
                            tt_new = wp.tile([N, N], F32R, tag=f"TT_{tag}",
                                             bufs=2, name=f"TT_{tag}")
                            nc.vector.tensor_copy(t_new[:], psa[:])
                            nc.scalar.copy(tt_new[:], psb[:])
                            t_cur[tag], tt_cur[tag] = t_new, tt_new
                    return tt_cur

                eres = expm_batch([
                    (zt["ZC_U"], "UCT"), (zt["ZC_V"], "VCT"),
                    (zt["ZB_U"], "UBT"), (zt["ZB_V"], "VBT"),
                    (zt["ZA_U"], "UAT"),
                ])
                uct, vct = eres["UCT"], eres["VCT"]
                ubt, vbt = eres["UBT"], eres["VBT"]
                uat = eres["UAT"]

                def absdiag_col(z_tile, tag):
                    tmp = wp.tile([N, N], F32, tag="us")
                    nc.vector.tensor_mul(tmp[:], z_tile[:], ident[:])
                    col = sp.tile([N, 1], F32, tag=tag, name=f"col_{tag}")
                    nc.vector.tensor_reduce(
                        col[:], tmp[:], AXIS.X, ALU.add, apply_absolute_value=True
                    )
                    return col

                dc_col = absdiag_col(zt["ZC_S"], "dc")
                db_col = absdiag_col(zt["ZB_S"], "db")
                ga_col = absdiag_col(zt["ZA_G"], "ga")

                # top-4: alpha = sqrt(sum_i (b_i c_i)^2), b/c sorted desc.
                bwork = sp.tile([N, 1], F32, tag="bwork")
                cwork = sp.tile([N, 1], F32, tag="cwork")
                nc.vector.tensor_copy(bwork[:], db_col[:])
                nc.vector.tensor_copy(cwork[:], dc_col[:])
                acc = sp.tile([N, 1], F32, tag="acc")
                nc.vector.memset(acc[:], 0.0)
                bmax = sp.tile([N, 1], F32, tag="bmax")
                cmax = sp.tile([N, 1], F32, tag="cmax")
                prod = sp.tile([N, 1], F32, tag="prod")
                gmask = sp.tile([N, 1], F32, tag="gmask")
                tdrop = sp.tile([N, 1], F32, tag="tdrop")
                for i in range(KTOP):
                    nc.gpsimd.partition_all_reduce(
                        bmax[:], bwork[:], N, bass_isa.ReduceOp.max
                    )
                    nc.gpsimd.partition_all_reduce(
                        cmax[:], cwork[:], N, bass_isa.ReduceOp.max
                    )
                    nc.vector.tensor_mul(prod[:], bmax[:], cmax[:])
                    nc.vector.tensor_mul(prod[:], prod[:], prod[:])
                    nc.vector.tensor_add(acc[:], acc[:], prod[:])
                    if i < KTOP - 1:
                        nc.vector.tensor_single_scalar(
                            gmask[:], bwork[:], bmax[:], ALU.is_ge
                        )
                        nc.vector.tensor_mul(tdrop[:], bwork[:], gmask[:])
                        nc.vector.tensor_sub(bwork[:], bwork[:], tdrop[:])
                        nc.vector.tensor_single_scalar(
                            gmask[:], cwork[:], cmax[:], ALU.is_ge
                        )
                        nc.vector.tensor_mul(tdrop[:], cwork[:], gmask[:])
                        nc.vector.tensor_sub(cwork[:], cwork[:], tdrop[:])
                alpha = sp.tile([N, 1], F32, tag="alpha")
                nc.scalar.activation(alpha[:], acc[:], ACTF.Sqrt)

                sa05 = sp.tile([N, 1], F32, tag="sa05")
                nc.vector.tensor_scalar(
                    sa05[:], ga_col[:], alpha[:], -0.5, op0=ALU.add, op1=ALU.mult
                )
                sb01 = sp.tile([N, 1], F32, tag="sb01")
                nc.vector.tensor_scalar_mul(sb01[:], db_col[:], STEP)

                # C^T = VC @ (SC @ UC^T)
                p1 = wp.tile([N, N], F32R, tag="us_r", name="p1")
                nc.vector.tensor_scalar_mul(p1[:], uct[:], dc_col[:])
                psa = psp.tile([N, N], F32, tag="ps", bufs=8)
                nc.tensor.matmul(psa[:], vct[:], p1[:], start=True, stop=True)
                nc.vector.tensor_copy(CTf32[:], psa[:])

                # G^T = (0.01 B)^T = VB @ (0.01 SB @ UB^T)
                p2 = wp.tile([N, N], F32R, tag="us_r", name="p2")
                nc.vector.tensor_scalar_mul(p2[:], ubt[:], sb01[:])
                psb = psp.tile([N, N], F32, tag="ps", bufs=8)
                nc.tensor.matmul(psb[:], vbt[:], p2[:], start=True, stop=True)
                nc.vector.tensor_copy(BpTm[:], psb[:])
                W1T = ep.tile([N, N], F32, tag="W1T")      # G^T fp32
                nc.scalar.copy(W1T[:], psb[:])
                # untransposed G = 0.01 B = UB @ (0.01 SB @ VB^T)
                p2b = wp.tile([N, N], F32R, tag="us_r", name="p2b")
                nc.vector.tensor_scalar_mul(p2b[:], vbt[:], sb01[:])
                psb2 = psp.tile([N, N], F32, tag="ps", bufs=8)
                nc.tensor.matmul(psb2[:], ubt[:], p2b[:], start=True, stop=True)
                bp_un = ep.tile([N, N], F32, tag="Bpun")
                nc.vector.tensor_copy(bp_un[:], psb2[:])

                # A: M = UA @ (sa05 * UA^T); YA part via masked transpose
                p3 = wp.tile([N, N], F32R, tag="us_r", name="p3")
                nc.vector.tensor_scalar_mul(p3[:], uat[:], sa05[:])
                psm = psp.tile([N, N], F32, tag="ps", bufs=8)
                nc.tensor.matmul(psm[:], uat[:], p3[:], start=True, stop=True)
                uy = wp.tile([N, N], F32, tag="us")
                nc.vector.tensor_mul(uy[:], zt["ZA_Y"][:], masku[:])
                pst2 = psp.tile([N, N], F32, tag="ps", bufs=8)
                nc.tensor.transpose(pst2[:], uy[:], ident[:])
                nc.vector.tensor_scalar_mul(uy[:], uy[:], 0.5 * STEP)
                q2 = wp.tile([N, N], F32, tag="T")
                nc.vector.scalar_tensor_tensor(
                    q2[:], pst2[:], 0.5 * STEP, uy[:], op0=ALU.mult, op1=ALU.subtract
                )
                # (0.01 A)^T fp32 + fp16; untransposed 0.01 A fp32
                A01Tf = ep.tile([N, N], F32, tag="A01Tf")
                nc.vector.scalar_tensor_tensor(
                    A01Tf[:], psm[:], STEP, q2[:], op0=ALU.mult, op1=ALU.add
                )
                nc.vector.tensor_copy(A01Tm[:], A01Tf[:])
                a01_un = ep.tile([N, N], F32, tag="A01un")
                nc.vector.scalar_tensor_tensor(
                    a01_un[:], psm[:], STEP, q2[:], op0=ALU.mult, op1=ALU.subtract
                )

                def lmul_AtT(x_tile, out_tile, tagname):
                    """out = At^T @ x = x + (0.01A)^T x   (fp32 tiles)."""
                    ps = psp.tile([N, N], F32, tag="ps", bufs=8, name=f"ps_{tagname}")
                    nc.tensor.matmul(ps[:], a01_un[:], x_tile[:], start=True, stop=True)
                    nc.vector.scalar_tensor_tensor(
                        out_tile[:], ps[:], 1.0, x_tile[:], op0=ALU.mult, op1=ALU.add
                    )

                def lmul_A01T(x_tile, out_tile, tagname):
                    """out = (0.01A)^T @ x   (fp32 tiles)."""
                    ps = psp.tile([N, N], F32, tag="ps", bufs=8, name=f"psA_{tagname}")
                    nc.tensor.matmul(ps[:], a01_un[:], x_tile[:], start=True, stop=True)
                    nc.vector.tensor_copy(out_tile[:], ps[:])

                # T1..T4: (At^k)^T C^T chain; P4^T = T4
                T1 = ep.tile([N, N], F32, tag="T1")
                lmul_AtT(CTf32, T1, "T1")
                T2 = ep.tile([N, N], F32, tag="T2")
                lmul_AtT(T1, T2, "T2")
                T3 = ep.tile([N, N], F32, tag="T3")
                lmul_AtT(T2, T3, "T3")
                T4 = ep.tile([N, N], F32, tag="T4")
                lmul_AtT(T3, T4, "T4")
                nc.vector.tensor_copy(P4Tm[:], T4[:])

                # Q4^T = G^T (C (I+At+At^2+At^3))^T = G^T (C^T+T1+T2+T3)
                ssum = wp.tile([N, N], F32, tag="us")
                nc.vector.tensor_add(ssum[:], CTf32[:], T1[:])
                nc.vector.tensor_add(ssum[:], ssum[:], T2[:])
                nc.vector.tensor_add(ssum[:], ssum[:], T3[:])
                psq4 = psp.tile([N, N], F32, tag="ps", bufs=8)
                nc.tensor.matmul(psq4[:], bp_un[:], ssum[:], start=True, stop=True)
                nc.vector.tensor_copy(Q4Tm[:], psq4[:])

                # A4^T = (0.01A At^3)^T = (At^T)^3 (0.01A)^T
                U1 = ep.tile([N, N], F32, tag="U1")
                lmul_AtT(A01Tf, U1, "U1")
                U2 = ep.tile([N, N], F32, tag="U2")
                lmul_AtT(U1, U2, "U2")
                U3 = ep.tile([N, N], F32, tag="U3")
                lmul_AtT(U2, U3, "U3")
                nc.vector.tensor_copy(A4Tm[:], U3[:])

                # W_j^T: u_{t-j} coefficients of delta (j=1..4):
                #   W1 = G, W2 = 0.01A G, W3 = 0.01A At G, W4 = 0.01A At^2 G
                # W_j^T = G^T (At^{j-2})^T (0.01A)^T for j>=2
                W2T = ep.tile([N, N], F32, tag="W2T")
                psw2 = psp.tile([N, N], F32, tag="ps", bufs=8)
                nc.tensor.matmul(psw2[:], bp_un[:], A01Tf[:], start=True, stop=True)
                nc.vector.tensor_copy(W2T[:], psw2[:])
                W3T = ep.tile([N, N], F32, tag="W3T")
                psw3 = psp.tile([N, N], F32, tag="ps", bufs=8)
                nc.tensor.matmul(psw3[:], bp_un[:], U1[:], start=True, stop=True)
                nc.vector.tensor_copy(W3T[:], psw3[:])
                W4T = ep.tile([N, N], F32, tag="W4T")
                psw4 = psp.tile([N, N], F32, tag="ps", bufs=8)
                nc.tensor.matmul(psw4[:], bp_un[:], U2[:], start=True, stop=True)
                nc.vector.tensor_copy(W4T[:], psw4[:])

                # GA = sum_j W_j*(1+(4-j)/4); GB = -sum_j W_j*(4-j)/4
                # G4 (plain) = sum_j W_j
                acc1 = wp.tile([N, N], F32, tag="us")
                nc.vector.tensor_scalar_mul(acc1[:], W1T[:], 1.75)
                nc.vector.scalar_tensor_tensor(
                    acc1[:], W2T[:], 1.5, acc1[:], op0=ALU.mult, op1=ALU.add
                )
                nc.vector.scalar_tensor_tensor(
                    acc1[:], W3T[:], 1.25, acc1[:], op0=ALU.mult, op1=ALU.add
                )
                nc.vector.scalar_tensor_tensor(
                    acc1[:], W4T[:], 1.0, acc1[:], op0=ALU.mult, op1=ALU.add
                )
                nc.vector.tensor_copy(GATm[:], acc1[:])
                acc2 = wp.tile([N, N], F32, tag="T")
                nc.vector.tensor_scalar_mul(acc2[:], W1T[:], -0.75)
                nc.vector.scalar_tensor_tensor(
                    acc2[:], W2T[:], -0.5, acc2[:], op0=ALU.mult, op1=ALU.add
                )
                nc.vector.scalar_tensor_tensor(
                    acc2[:], W3T[:], -0.25, acc2[:], op0=ALU.mult, op1=ALU.add
                )
                nc.vector.tensor_copy(GBTm[:], acc2[:])
                acc3 = wp.tile([N, N], F32, tag="us", name="acc3")
                nc.vector.tensor_add(acc3[:], W1T[:], W2T[:])
                nc.vector.tensor_add(acc3[:], acc3[:], W3T[:])
                nc.vector.tensor_add(acc3[:], acc3[:], W4T[:])
                nc.vector.tensor_copy(G4Tm[:], acc3[:])

                # vectors: c = 0.01 bx; w1 = At c, w2 = At w1, w3 = At w2
                # r4 = C (c+w1+w2+w3) + by
                # cc = c + 0.01A (c+w1+w2)
                def atv(v_in, tagname):
                    ps = psp.tile([N, 1], F32, tag="ps", bufs=8, name=f"pv_{tagname}")
                    nc.tensor.matmul(ps[:], A01Tf[:], v_in[:], start=True, stop=True)
                    v_out = sp.tile([N, 1], F32, tag=tagname)
                    nc.vector.scalar_tensor_tensor(
                        v_out[:], ps[:], 1.0, v_in[:], op0=ALU.mult, op1=ALU.add
                    )
                    return v_out

                w1 = atv(bxp_c, "w1")
                w2 = atv(w1, "w2")
                w3 = atv(w2, "w3")
                vs = sp.tile([N, 1], F32, tag="vs")
                nc.vector.tensor_add(vs[:], bxp_c[:], w1[:])
                vs2 = sp.tile([N, 1], F32, tag="vs2")
                nc.vector.tensor_add(vs2[:], vs[:], w2[:])
                vs3 = sp.tile([N, 1], F32, tag="vs3")
                nc.vector.tensor_add(vs3[:], vs2[:], w3[:])
                psr4 = psp.tile([N, 1], F32, tag="ps", bufs=8, name="psr4")
                nc.tensor.matmul(psr4[:], CTf32[:], vs3[:], start=True, stop=True)
                nc.vector.scalar_tensor_tensor(
                    r4_c[:], psr4[:], 1.0, by_c[:], op0=ALU.mult, op1=ALU.add
                )
                pscc = psp.tile([N, 1], F32, tag="ps", bufs=8, name="pscc")
                nc.tensor.matmul(pscc[:], A01Tf[:], vs2[:], start=True, stop=True)
                nc.vector.scalar_tensor_tensor(
                    cc_c[:], pscc[:], 1.0, bxp_c[:], op0=ALU.mult, op1=ALU.add
                )
                cc2_c = constp.tile([N, 1], F32, tag="cc2c")
                nc.vector.tensor_scalar_mul(cc2_c[:], cc_c[:], 2.0)

            # ------- recurrence: lag-4, paired steps -------
            with (
                tc.tile_pool(name="xrb", bufs=2) as xrbp,
                tc.tile_pool(name="ub", bufs=2) as ubp,
                tc.tile_pool(name="xb", bufs=4) as xbp,
                tc.tile_pool(name="dx", bufs=3) as dxp,
                tc.tile_pool(name="psy", bufs=3, space="PSUM") as psyp,
                tc.tile_pool(name="psx", bufs=3, space="PSUM") as psxp,
            ):
                CH = tc_chunk
                nchunks = tmax // CH

                xrb = xrbp.tile([N, CH * BSH], mdt, tag="xrb")
                ub = ubp.tile([N, CH * BSH], mdt, tag="ub")
                xr_bufs = {0: xrb}
                ub_bufs = {0: ub}

                # ---- bootstrap steps 0..3 (exact per-step form) ----
                nc.vector.tensor_copy(xrb[:, 0:BSH], x0_c[:])
                xb_cur = x0_c
                for k in range(4):
                    psyb = psyp.tile([N, 2 * BSH], F32, tag="psy", name=f"psyb{k}")
                    nc.tensor.matmul(
                        psyb[:, 0:BSH], CTf32[:], xb_cur[:], start=True, stop=True
                    )
                    nc.scalar.activation(
                        ub[:, ds(k * BSH, BSH)], psyb[:, 0:BSH], ACTF.Tanh,
                        bias=by_c[:], scale=1.0,
                    )
                    if k < 3:
                        psxb = psxp.tile([N, 2 * BSH], F32, tag="psx",
                                         name=f"psxb{k}")
                        nc.tensor.matmul(
                            psxb[:, 0:BSH], A01Tm[:], xrb[:, ds(k * BSH, BSH)],
                            start=True, stop=False,
                        )
                        nc.tensor.matmul(
                            psxb[:, 0:BSH], BpTm[:], ub[:, ds(k * BSH, BSH)],
                            start=False, stop=True,
                        )
                        xb_new = xbp.tile([N, BSH], F32, tag="xb", name=f"xbb{k}")
                        nc.vector.scalar_tensor_tensor(
                            xb_new[:], psxb[:, 0:BSH], bxp_c[:], xb_cur[:],
                            op0=ALU.add, op1=ALU.add,
                        )
                        nc.vector.scalar_tensor_tensor(
                            xrb[:, ds((k + 1) * BSH, BSH)], psxb[:, 0:BSH],
                            bxp_c[:], xb_cur[:], op0=ALU.add, op1=ALU.add,
                        )
                        xb_cur = xb_new
                # xb_cur == x_3 (the odd-step fp32 carry)

                # ---- main pair loop: t = 4, 6, ..., tmax-2 ----
                for t in range(4, tmax, 2):
                    s = t % CH
                    if s == 0:
                        cidx = t // CH
                        xrb = xrbp.tile([N, CH * BSH], mdt, tag="xrb")
                        ub = ubp.tile([N, CH * BSH], mdt, tag="ub")
                        xr_bufs[cidx] = xrb
                        ub_bufs[cidx] = ub
                        xr_bufs.pop(cidx - 2, None)
                        ub_bufs.pop(cidx - 2, None)

                    def pslice(bufs, tt):
                        b = bufs[tt // CH]
                        return b[:, ds((tt % CH) * BSH, 2 * BSH)]

                    xr4 = pslice(xr_bufs, t - 4)
                    u4 = pslice(ub_bufs, t - 4)

                    # psx pair: columns [delta_t - cc | delta_{t+1} - cc]
                    psx = psxp.tile([N, 2 * BSH], F32, tag="psx")
                    if t >= 8:
                        u8 = pslice(ub_bufs, t - 8)
                        nc.tensor.matmul(psx[:], GATm[:], u4, start=True, stop=False)
                        nc.tensor.matmul(psx[:], GBTm[:], u8, start=False, stop=False)
                        nc.tensor.matmul(psx[:], A4Tm[:], xr4, start=False, stop=True)
                    else:
                        nc.tensor.matmul(psx[:], G4Tm[:], u4, start=True, stop=False)
                        nc.tensor.matmul(psx[:], A4Tm[:], xr4, start=False, stop=True)
                    # psy pair -> tanh pair
                    psy = psyp.tile([N, 2 * BSH], F32, tag="psy")
                    nc.tensor.matmul(psy[:], Q4Tm[:], u4, start=True, stop=False)
                    nc.tensor.matmul(psy[:], P4Tm[:], xr4, start=False, stop=True)

                    # ACT: delta_t evacuation (x-loop critical), then tanh pair
                    # (cc is folded into the DVE ops: Copy takes no AP bias)
                    dxe = dxp.tile([N, BSH], F32, tag="dxe")
                    nc.scalar.copy(dxe[:], psx[:, 0:BSH])
                    nc.scalar.activation(
                        ub[:, ds(s * BSH, 2 * BSH)], psy[:], ACTF.Tanh,
                        bias=r4_c[:], scale=1.0,
                    )

                    # DVE: x_t (fp16), pair-sum s2, x_{t+1} (fp16)
                    nc.vector.scalar_tensor_tensor(
                        xrb[:, ds(s * BSH, BSH)], dxe[:], cc_c[:], xb_cur[:],
                        op0=ALU.add, op1=ALU.add,
                    )
                    s2 = dxp.tile([N, BSH], F32, tag="s2")
                    nc.vector.scalar_tensor_tensor(
                        s2[:], psx[:, ds(BSH, BSH)], cc2_c[:], dxe[:],
                        op0=ALU.add, op1=ALU.add,
                    )
                    nc.vector.tensor_add(
                        xrb[:, ds((s + 1) * BSH, BSH)], s2[:], xb_cur[:]
                    )
                    # GPSIMD: fp32 pair carry (SBUF-only)
                    xb_new = xbp.tile([N, BSH], F32, tag="xb")
                    nc.gpsimd.tensor_add(xb_new[:], s2[:], xb_cur[:])
                    xb_cur = xb_new

                    if s + 2 == CH:
                        c = t // CH
                        nc.sync.dma_start(
                            out=out[:, ds(c * CH * BSH, CH * BSH)],
                            in_=xrb[:, 0:CH * BSH],
                        )

    nc.compile()
    return nc


_CACHED = {}


def _get_program(tmax=TMAX, tc_chunk=64, mdt=FP16):
    key = (tmax, tc_chunk, str(mdt))
    if key not in _CACHED:
        _CACHED[key] = build_program(tmax, tc_chunk, mdt)
    return _CACHED[key]


def make_in_maps(inputs, tmax=TMAX):
    X0 = np.ascontiguousarray(np.asarray(inputs["X0"], dtype=np.float32))
    base = {
        name: np.ascontiguousarray(np.asarray(inputs[name], dtype=np.float32))
        for name in PARAM_NAMES
    }
    base["bx"] = np.ascontiguousarray(
        np.asarray(inputs["bx"], dtype=np.float32).reshape(N, 1)
    )
    base["by"] = np.ascontiguousarray(
        np.asarray(inputs["by"], dtype=np.float32).reshape(N, 1)
    )
    in_maps = []
    for c in range(NCORES):
        m = dict(base)
        m["x0"] = np.ascontiguousarray(X0[c * BSH:(c + 1) * BSH].T)
        in_maps.append(m)
    return in_maps


def run_spmd(inputs, tmax=TMAX, tc_chunk=64, trace=False, tmpdir=None, mdt=FP16):
    nc = _get_program(tmax, tc_chunk, mdt)
    in_maps = make_in_maps(inputs, tmax)
    res = run_bass_kernel_spmd(
        nc, in_maps, list(range(NCORES)), trace=trace, tmpdir=tmpdir
    )
    X0 = np.asarray(inputs["X0"], dtype=np.float32)
    outs = []
    for c in range(NCORES):
        o = np.asarray(res.results[c]["out"])        # [N, tmax*BSH] fp16
        o = o.reshape(N, tmax, BSH).transpose(2, 1, 0).astype(np.float32)
        outs.append(o)                               # (BSH, tmax, N)
    full = np.concatenate(outs, axis=0)              # (BS, tmax, N)
    full[:, 0, :] = X0                               # exact t=0 plane
    return full, res


def kernel(**inputs):
    full, _ = run_spmd(inputs)
    return full
